# revision 41
# baseline (speedup 1.0000x reference)
"""Trainium2 Bass kernel for nn_AttnResBlockUp (B=16, IN=512, OUT=256, H=W=32, L=32).

Sharding: data-parallel over batch (2 items per core, 8 cores); BatchNorm
batch stats sync'd with a cross-core AllReduce (sync-BN).

v2 design (vs the f32r/spill baseline):
  - all matmul operands bf16 (rel err budget 2e-2; emulated bf16 error 3.4e-3),
    DMA'd directly from host-converted arrays: no on-chip casts.
  - x and the conv1 output stay SBUF-resident: no DRAM spill, x loaded once.
  - words l2-normalized on host; the softmax mask folded into the exp bias
    (-30 for masked tokens); softmax denominator via gpsimd partition_all_reduce
    (Pool engine) instead of a PE matmul + broadcast.
  - 1/||img|| via Ln/Exp so every Act function {Ln,Exp,Relu,Copy,Identity,
    Square} lives in one activation table: 1 table load total.
  - pads zero only their borders (interior fully overwritten by relu sinks).
  - conv weights streamed once (q-major over both batch items).
  - BN1 stats from SBUF x (DVE reduce + Act square-accum), BN2 stats fused
    into the conv1 PSUM evacuation.
"""
import sys
sys.path.insert(0, "/opt/trn_rl_repo")

import contextlib
import numpy as np
import concourse.bass as bass
import concourse.bacc as bacc
import concourse.bass_isa as bass_isa
import concourse.mybir as mybir
import concourse.tile as tile

F32 = mybir.dt.float32
BF = mybir.dt.bfloat16
F8 = mybir.dt.float8e4
AX = mybir.AxisListType
OP = mybir.AluOpType
ACT = mybir.ActivationFunctionType

B, IN, OUT, GD, TD, H, W, L = 16, 512, 256, 256, 256, 32, 32, 32
COND = GD + TD
EPS_BN = 1e-5
N_CORES = 8
B_LOC = B // N_CORES
P1 = H * W                    # 1024
P2 = 4 * P1                   # 4096
CHUNK = 512
KT1 = IN // 128               # 4
KT2 = OUT // 128              # 2
MT = TD // 128                # 2
NCH1 = P1 // CHUNK            # 2
NCH2 = P2 // CHUNK            # 8
PW1, PW2 = 34, 66
MASK_BIAS = -30.0


def build_program(num_devices=N_CORES, use_collectives=True):
    nc = bacc.Bacc("TRN2", target_bir_lowering=False, debug=False,
                   num_devices=num_devices)

    def din(name, shape, dt=BF):
        return nc.dram_tensor(name, list(shape), dt, kind="ExternalInput")

    x_d = din("x", (B_LOC, IN, P1))
    gc_d = din("gcT", (GD, B_LOC))
    wordsn_d = din("wordsn", (B_LOC, TD, L))
    wordsTn_d = din("wordsTn", (B_LOC, L, TD))
    maskb_d = din("maskb", (B_LOC, L), F32)
    wimg1_d = din("w_img1T", (IN, TD))
    wimg2_d = din("w_img2T", (OUT, TD))
    wg1_d = din("w_g1T", (COND, IN))
    wb1_d = din("w_b1T", (COND, IN))
    wg2_d = din("w_g2T", (COND, OUT))
    wb2_d = din("w_b2T", (COND, OUT))
    bg1_d = din("b_g1", (IN,), F32)
    bb1_d = din("b_b1", (IN,), F32)
    bg2_d = din("b_g2", (OUT,), F32)
    bb2_d = din("b_b2", (OUT,), F32)
    w1h_d = din("w1drh", (4, 4, 2, 2, 128, 2, 128), F8)  # [q][t][kp][mo][c][s][m] hi
    w1l_d = din("w1drl", (4, 4, 2, 2, 128, 2, 128), F8)  # lo
    w2h_d = din("w2drh", (9, 2, 128, 2, 128), F8)  # [t][mo][c][slot][m] hi
    w2l_d = din("w2drl", (9, 2, 128, 2, 128), F8)  # lo
    wsc_d = din("w_scT", (IN, OUT))
    bsc_d = din("b_sc", (OUT,), F32)
    bn1w_d = din("bn1_w", (IN,), F32)
    bn1b_d = din("bn1_b", (IN,), F32)
    bn2w_d = din("bn2_w", (OUT,), F32)
    bn2b_d = din("bn2_b", (OUT,), F32)

    out_d = nc.dram_tensor("out", [B_LOC, OUT, P2], F32, kind="ExternalOutput")

    with tile.TileContext(nc) as tc:
        st = contextlib.ExitStack()
        cpool = st.enter_context(tc.tile_pool(name="cpool", bufs=1))
        scr = st.enter_context(tc.tile_pool(name="scr", bufs=2))
        dram = st.enter_context(tc.tile_pool(name="dram", bufs=1, space="DRAM"))
        p1b = contextlib.ExitStack()
        ph1b = p1b.enter_context(tc.tile_pool(name="ph1b", bufs=1))
        psum_st = contextlib.ExitStack()
        psum = psum_st.enter_context(tc.tile_pool(name="psum1", bufs=1, space="PSUM"))
        p1pads_st = contextlib.ExitStack()
        p1pads = p1pads_st.enter_context(tc.tile_pool(name="p1pads", bufs=1))

        # ---------------- constants ----------------
        onesf = cpool.tile([128, 1], F32, name="onesf")
        nc.vector.memset(onesf[:], 1.0)
        ones_bf = cpool.tile([128, 1], BF, name="ones_bf")
        nc.vector.tensor_copy(ones_bf[:], onesf[:])
        eps_t = cpool.tile([128, 1], F32, name="eps_t")
        nc.vector.memset(eps_t[:], float(EPS_BN))
        zrow_bf = cpool.tile([128, 1], BF, name="zrow_bf")
        nc.vector.memset(zrow_bf[:], 0.0)

        def load_vec(dram_t, c, name):
            kt = c // 128
            t = cpool.tile([128, kt], F32, name=name)
            nc.sync.dma_start(t[:], dram_t.ap().rearrange("(k p) -> p k", p=128))
            return t

        # ---------------- resident inputs + ALL weights (prologue DMAs) ------
        x_t = [p1pads.tile([128, KT1 * P1], BF, name=f"x_{b}") for b in range(B_LOC)]
        x_sb = [[x_t[b][:, k * P1:(k + 1) * P1] for k in range(KT1)] for b in range(B_LOC)]
        for b in range(B_LOC):
            nc.sync.dma_start(x_t[b][:].rearrange("p (k c) -> p k c", k=KT1),
                              x_d.ap()[b].rearrange("(k p) c -> p k c", p=128))

        words_t = cpool.tile([128, B_LOC * MT * L], BF, name="words_t")
        nc.sync.dma_start(words_t[:].rearrange("p (b m l) -> p b m l", b=B_LOC, m=MT),
                          wordsn_d.ap().rearrange("b (m p) l -> p b m l", p=128))
        words_sb = [[words_t[:, (b * MT + m) * L:(b * MT + m + 1) * L] for m in range(MT)]
                    for b in range(B_LOC)]
        wordsT_t = cpool.tile([L, B_LOC * TD], BF, name="wordsT_t")
        nc.sync.dma_start(wordsT_t[:].rearrange("p (b c) -> p b c", b=B_LOC),
                          wordsTn_d.ap().rearrange("b p c -> p b c"))
        wordsT_sb = [wordsT_t[:, b * TD:(b + 1) * TD] for b in range(B_LOC)]
        maskb_sb = cpool.tile([L, B_LOC], F32, name="maskb_sb")
        nc.sync.dma_start(maskb_sb[:], maskb_d.ap().rearrange("b l -> l b"))
        gc_sb = [cpool.tile([128, B_LOC], BF, name=f"gc_{k}") for k in range(GD // 128)]
        for k in range(GD // 128):
            nc.sync.dma_start(gc_sb[k][:], gc_d.ap()[k * 128:(k + 1) * 128, :])

        bg1 = load_vec(bg1_d, IN, "bg1")
        bb1 = load_vec(bb1_d, IN, "bb1")
        bg2 = load_vec(bg2_d, OUT, "bg2")
        bb2 = load_vec(bb2_d, OUT, "bb2")
        bsc = load_vec(bsc_d, OUT, "bsc")
        bn1w = load_vec(bn1w_d, IN, "bn1w")
        bn1b = load_vec(bn1b_d, IN, "bn1b")
        bn2w = load_vec(bn2w_d, OUT, "bn2w")
        bn2b = load_vec(bn2b_d, OUT, "bn2b")

        def wload(dram_t, kt, width, name):
            t = cpool.tile([128, kt * width], BF, name=name)
            nc.sync.dma_start(t[:].rearrange("p (k c) -> p k c", c=width),
                              dram_t.ap().rearrange("(k p) c -> p k c", p=128))
            return [t[:, k * width:(k + 1) * width] for k in range(kt)]

        wimg1 = wload(wimg1_d, KT1, TD, "wimg1")
        wsc = wload(wsc_d, KT1, OUT, "wsc")
        wg1 = wload(wg1_d, COND // 128, IN, "wg1")
        wb1 = wload(wb1_d, COND // 128, IN, "wb1")
        wimg2 = wload(wimg2_d, KT2, TD, "wimg2")
        wg2 = wload(wg2_d, COND // 128, OUT, "wg2")
        wb2 = wload(wb2_d, COND // 128, OUT, "wb2")
        w2h_t = cpool.tile([128, 9 * 2 * 2 * 128], F8, name="w2h_t")
        nc.sync.dma_start(w2h_t[:].rearrange("p (t mo s m) -> p t mo s m", t=9, mo=2, s=2),
                          w2h_d.ap().rearrange("t mo p s m -> p t mo s m"))
        w2l_t = cpool.tile([128, 9 * 2 * 2 * 128], F8, name="w2l_t")
        nc.sync.dma_start(w2l_t[:].rearrange("p (t mo s m) -> p t mo s m", t=9, mo=2, s=2),
                          w2l_d.ap().rearrange("t mo p s m -> p t mo s m"))

        def w2dr(ht, t, mo):
            return ht[:, (t * 2 + mo) * 256:(t * 2 + mo) * 256 + 256].rearrange(
                "p (s m) -> p s m", s=2)

        # long-lived activations
        sc_sb = [[ph1b.tile([128, P1], BF, name=f"sc_{b}_{m}") for m in range(KT2)]
                 for b in range(B_LOC)]
        pads1H = [p1pads.tile([128, KT1 * PW1 * PW1], F8, name=f"pad1H_{b}")
                  for b in range(B_LOC)]
        pads1L = [p1pads.tile([128, KT1 * PW1 * PW1], F8, name=f"pad1L_{b}")
                  for b in range(B_LOC)]
        out1_d = dram.tile([B_LOC, OUT, P2], BF, name="out1_spill")
        # per-chunk norm rows (partition 0; engine ops must start at partition 0)
        n2row = ph1b.tile([1, P2], F32, name="n2row")
        invnrow = ph1b.tile([1, P2], F32, name="invnrow")
        sims = ph1b.tile([L, P2], BF, name="sims")

        zrow8 = cpool.tile([128, 1], F8, name="zrow8")
        nc.vector.memset(zrow8[:], 0.0)

        def zero_borders8(pt, k, kt, pw):
            v = pt[:].rearrange("p (k r c) -> p k r c", k=kt, r=pw)[:, k]
            zb = zrow8[:].broadcast_to((128, pw))
            nc.vector.tensor_copy(v[:, 0, :], zb)
            nc.vector.tensor_copy(v[:, pw - 1, :], zb)
            nc.vector.tensor_copy(v[:, :, 0:1].rearrange("p r 1 -> p r"), zb)
            nc.vector.tensor_copy(v[:, :, pw - 1:pw].rearrange("p r 1 -> p r"), zb)

        for b in range(B_LOC):
            for k in range(KT1):
                zero_borders8(pads1H[b], k, KT1, PW1)
                zero_borders8(pads1L[b], k, KT1, PW1)

        # ---------------- pass 0: BN1 partial stats from SBUF x --------------
        ar1_in = dram.tile([KT1, 128, 2], F32, name="ar1_in")
        ar1_out = dram.tile([KT1, 128, 2], F32, name="ar1_out",
                            addr_space="Shared" if use_collectives else "Local")
        st1 = cpool.tile([128, 2 * KT1], F32, name="st1")
        sumP = cpool.tile([128, 2 * KT1], F32, name="sumP")
        ssP = cpool.tile([128, 2 * KT1], F32, name="ssP")
        for k in range(KT1):
            for b in range(B_LOC):
                nc.vector.tensor_reduce(sumP[:, 2 * k + b:2 * k + b + 1],
                                        x_sb[b][k], AX.X, OP.add)
                thr = scr.tile([128, P1], BF, name=f"thr0_{k}_{b}", tag="scr_sq", bufs=2)
                nc.scalar.activation(thr[:], x_sb[b][k], ACT.Square,
                                     accum_out=ssP[:, 2 * k + b:2 * k + b + 1])
            nc.vector.tensor_reduce(st1[:, 2 * k:2 * k + 1],
                                    sumP[:, 2 * k:2 * k + 2], AX.X, OP.add)
            nc.vector.tensor_reduce(st1[:, 2 * k + 1:2 * k + 2],
                                    ssP[:, 2 * k:2 * k + 2], AX.X, OP.add)
            nc.sync.dma_start(ar1_in[k], st1[:, 2 * k:2 * k + 2])
        if use_collectives:
            nc.gpsimd.collective_compute(
                "AllReduce", OP.add, replica_groups=[list(range(num_devices))],
                ins=[ar1_in.opt()], outs=[ar1_out.opt()])
        else:
            nc.sync.dma_start(ar1_out[:], ar1_in[:])

        # ---------------- matvec A/B (global-cond part of gamma/beta) --------
        def matvec_AB(wg, wb, bgv, bbv, cout, name):
            mt = cout // 128
            A = cpool.tile([128, mt * B_LOC], F32, name=f"A_{name}")
            Bv = cpool.tile([128, mt * B_LOC], F32, name=f"B_{name}")
            for m in range(mt):
                pa = psum.tile([128, CHUNK], F32, name=f"pa_{name}_{m}", tag="ps_gb", bufs=2)[:, 0:B_LOC]
                for k in range(GD // 128):
                    nc.tensor.matmul(pa[:], wg[k][:, m * 128:(m + 1) * 128], gc_sb[k][:],
                                     start=(k == 0), stop=(k == GD // 128 - 1))
                nc.scalar.activation(A[:, m * B_LOC:(m + 1) * B_LOC], pa[:], ACT.Identity,
                                     bias=bgv[:, m:m + 1], scale=1.0)
                pb = psum.tile([128, CHUNK], F32, name=f"pb_{name}_{m}", tag="ps_gb", bufs=2)[:, 0:B_LOC]
                for k in range(GD // 128):
                    nc.tensor.matmul(pb[:], wb[k][:, m * 128:(m + 1) * 128], gc_sb[k][:],
                                     start=(k == 0), stop=(k == GD // 128 - 1))
                nc.scalar.activation(Bv[:, m * B_LOC:(m + 1) * B_LOC], pb[:], ACT.Identity,
                                     bias=bbv[:, m:m + 1], scale=1.0)
            return A, Bv


        A1, B1 = matvec_AB(wg1, wb1, bg1, bb1, IN, "1")
        A2, B2 = matvec_AB(wg2, wb2, bg2, bb2, OUT, "2")

        # ---------------- BN post: mean/var -> scale/shift --------------------
        def bn_post(ar_out_tile, kt, n_total, bnw, bnb, name):
            g = cpool.tile([128, 2 * kt], F32, name=f"g_{name}")
            s_t = cpool.tile([128, kt], F32, name=f"s_{name}")
            t_t = cpool.tile([128, kt], F32, name=f"t_{name}")
            tmp = cpool.tile([128, 4 * kt], F32, name=f"tmp_{name}")
            for k in range(kt):
                nc.sync.dma_start(g[:, 2 * k:2 * k + 2], ar_out_tile[k])
                mean = tmp[:, 4 * k:4 * k + 1]
                var = tmp[:, 4 * k + 1:4 * k + 2]
                nc.vector.tensor_scalar_mul(mean, g[:, 2 * k:2 * k + 1], 1.0 / n_total)
                nc.vector.scalar_tensor_tensor(var, mean, 0.0, mean, OP.add, OP.mult)
                nc.vector.scalar_tensor_tensor(var, g[:, 2 * k + 1:2 * k + 2], 1.0 / n_total,
                                               var, OP.mult, OP.subtract)
            # batched sqrt then reciprocal (one Act Sqrt op, one DVE recip op)
            std = tmp[:, 0:4 * kt].rearrange("p (k f) -> p k f", f=4)[:, :, 2]
            var_v = tmp[:, 0:4 * kt].rearrange("p (k f) -> p k f", f=4)[:, :, 1]
            nc.scalar.activation(std, var_v, ACT.Sqrt, bias=eps_t[:], scale=1.0)
            istd = tmp[:, 0:4 * kt].rearrange("p (k f) -> p k f", f=4)[:, :, 3]
            nc.vector.reciprocal(istd, std)
            for k in range(kt):
                mean = tmp[:, 4 * k:4 * k + 1]
                istd_k = tmp[:, 4 * k + 3:4 * k + 4]
                nc.vector.tensor_tensor(s_t[:, k:k + 1], istd_k, bnw[:, k:k + 1], OP.mult)
                nc.vector.tensor_tensor(t_t[:, k:k + 1], mean, s_t[:, k:k + 1], OP.mult)
                nc.vector.tensor_tensor(t_t[:, k:k + 1], bnb[:, k:k + 1], t_t[:, k:k + 1], OP.subtract)
            return s_t, t_t

        B_STATS = B if use_collectives else B_LOC

        # ================= stage front / mid / back =================
        def stage_front(b, n, kt_in, loader, wimg, extra, name, roff=0):
            xcols = loader(b, n, "f")
            if extra is not None:
                extra(n, xcols)
            ic = []
            sq = scr.tile([128, MT * CHUNK], BF, name=f"sq_{name}_{b}_{n}", tag="scr_sq", bufs=2)
            for m in range(MT):
                pim = psum.tile([128, CHUNK], F32, name=f"pim_{name}_{b}_{m}_{n}", tag="ps_mm", bufs=2)
                for k in range(kt_in):
                    nc.tensor.matmul(pim[:], wimg[k][:, m * 128:(m + 1) * 128],
                                     xcols[k], start=(k == 0), stop=(k == kt_in - 1))
                t = scr.tile([128, CHUNK], BF, name=f"ic_{name}_{b}_{m}_{n}", tag="scr_ic", bufs=4)
                nc.scalar.copy(t[:], pim[:])
                ic.append(t)
            nc.vector.tensor_tensor(sq[:, 0:CHUNK], ic[0][:], ic[0][:], OP.mult)
            nc.gpsimd.tensor_tensor(sq[:, CHUNK:2 * CHUNK], ic[1][:], ic[1][:], OP.mult)
            pn2 = psum.tile([L, CHUNK], F32, name=f"pn2_{name}_{b}_{n}", tag="ps_sim", bufs=2)[0:1, :]
            for m in range(MT):
                nc.tensor.matmul(pn2[:], ones_bf[:], sq[:, m * CHUNK:(m + 1) * CHUNK],
                                 start=(m == 0), stop=(m == MT - 1))
            nc.vector.tensor_copy(n2row[:, (roff + n) * CHUNK:(roff + n + 1) * CHUNK], pn2[:])
            psim = psum.tile([L, CHUNK], F32, name=f"psim_{name}_{b}_{n}", tag="ps_sim", bufs=2)
            for m in range(MT):
                nc.tensor.matmul(psim[:], words_sb[b][m], ic[m][:],
                                 start=(m == 0), stop=(m == MT - 1))
            nc.vector.tensor_copy(sims[:, (roff + n) * CHUNK:(roff + n + 1) * CHUNK], psim[:])

        def stage_mid(b, nch, name, roff=0):
            lo, w = roff * CHUNK, nch * CHUNK
            nc.scalar.activation(n2row[:, lo:lo + w], n2row[:, lo:lo + w], ACT.Sqrt,
                                 bias=0.0, scale=1.0)
            nc.vector.reciprocal(invnrow[:, lo:lo + w], n2row[:, lo:lo + w])

        def stage_back(b, n, wg, wb, A, Bv, s_v, t_v, loader, sink, gb_evac_acts, name,
                       roff=0):
            mt_out = A.shape[1] // B_LOC
            xcols = loader(b, n, "b")
            invn_b = scr.tile([L, CHUNK], F32, name=f"invnb_{name}_{b}_{n}", tag="scr_invnb", bufs=2)
            nc.gpsimd.partition_broadcast(invn_b[:], invnrow[:, (roff + n) * CHUNK:(roff + n + 1) * CHUNK], channels=L)
            tsim = scr.tile([L, CHUNK], BF, name=f"tsim_{name}_{b}_{n}", tag="scr_tsim", bufs=2)
            nc.gpsimd.tensor_tensor(tsim[:], sims[:, (roff + n) * CHUNK:(roff + n + 1) * CHUNK],
                                    invn_b[:], OP.mult)
            e_t = scr.tile([L, CHUNK], BF, name=f"e_{name}_{b}_{n}", tag="scr_e", bufs=2)
            nc.scalar.activation(e_t[:], tsim[:], ACT.Exp,
                                 bias=maskb_sb[:, b:b + 1], scale=1.0)
            denb = scr.tile([L, CHUNK], F32, name=f"denb_{name}_{b}_{n}", tag="scr_denb", bufs=2)
            nc.gpsimd.partition_all_reduce(denb[:], e_t[:], channels=L,
                                           reduce_op=bass_isa.ReduceOp.add)
            rden = scr.tile([L, CHUNK], F32, name=f"rden_{name}_{b}_{n}", tag="scr_rden", bufs=2)
            nc.vector.reciprocal(rden[:], denb[:])
            en = scr.tile([L, CHUNK], BF, name=f"en_{name}_{b}_{n}", tag="scr_en", bufs=2)
            nc.vector.tensor_tensor(en[:], e_t[:], rden[:], OP.mult)
            ctx = []
            for m in range(MT):
                pctx = psum.tile([128, CHUNK], F32, name=f"pctx_{name}_{b}_{m}_{n}", tag="ps_mm", bufs=2)
                nc.tensor.matmul(pctx[:], wordsT_sb[b][:, m * 128:(m + 1) * 128], en[:],
                                 start=True, stop=True)
                t = scr.tile([128, CHUNK], BF, name=f"ctx_{name}_{b}_{m}_{n}", tag="scr_ctx", bufs=4)
                nc.scalar.copy(t[:], pctx[:])
                ctx.append(t)
            for mo in range(mt_out):
                pg = psum.tile([128, CHUNK], F32, name=f"pg_{name}_{b}_{mo}_{n}", tag="ps_gb", bufs=2)
                for k in range(MT):
                    nc.tensor.matmul(pg[:], wg[GD // 128 + k][:, mo * 128:(mo + 1) * 128],
                                     ctx[k][:], start=(k == 0), stop=(k == MT - 1))
                pb = psum.tile([128, CHUNK], F32, name=f"pb_{name}_{b}_{mo}_{n}", tag="ps_gb", bufs=2)
                for k in range(MT):
                    nc.tensor.matmul(pb[:], wb[GD // 128 + k][:, mo * 128:(mo + 1) * 128],
                                     ctx[k][:], start=(k == 0), stop=(k == MT - 1))
                Acol = A[:, mo * B_LOC + b:mo * B_LOC + b + 1]
                Bcol = Bv[:, mo * B_LOC + b:mo * B_LOC + b + 1]
                xcol = xcols[mo]
                bnx = scr.tile([128, CHUNK], BF, name=f"bnx_{name}_{b}_{mo}_{n}", tag="scr_bnx", bufs=4)
                nc.vector.tensor_scalar(bnx[:], xcol, s_v[:, mo:mo + 1],
                                        t_v[:, mo:mo + 1], OP.mult, OP.add)
                pre = scr.tile([128, CHUNK], BF, name=f"pre_{name}_{b}_{mo}_{n}", tag="scr_pre", bufs=4)
                if mo < gb_evac_acts:
                    gs = scr.tile([128, CHUNK], BF, name=f"gs_{name}_{b}_{mo}_{n}", tag="scr_gs", bufs=3)
                    nc.scalar.activation(gs[:], pg[:], ACT.Identity, bias=Acol, scale=1.0)
                    bs = scr.tile([128, CHUNK], BF, name=f"bs_{name}_{b}_{mo}_{n}", tag="scr_bs", bufs=3)
                    nc.scalar.activation(bs[:], pb[:], ACT.Identity, bias=Bcol, scale=1.0)
                    t1 = scr.tile([128, CHUNK], BF, name=f"t1_{name}_{b}_{mo}_{n}", tag="scr_t1", bufs=4)
                    nc.vector.tensor_tensor(t1[:], gs[:], bnx[:], OP.mult)
                    nc.vector.tensor_tensor(pre[:], t1[:], bs[:], OP.add)
                else:
                    t1 = scr.tile([128, CHUNK], BF, name=f"t1_{name}_{b}_{mo}_{n}", tag="scr_t1", bufs=4)
                    nc.vector.scalar_tensor_tensor(t1[:], pg[:], Acol, bnx[:], OP.add, OP.mult)
                    nc.vector.scalar_tensor_tensor(pre[:], pb[:], Bcol, t1[:], OP.add, OP.add)
                sink(mo, n, pre[:])

        # ================= stage 1 (b-interleaved via row offsets) =============
        def extra1(b):
            def f(n, xcols, _b=b):
                for m in range(KT2):
                    ps = psum.tile([128, CHUNK], F32, name=f"psc_{_b}_{m}_{n}", tag="ps_gb", bufs=2)
                    for k in range(KT1):
                        nc.tensor.matmul(ps[:], wsc[k][:, m * 128:(m + 1) * 128], xcols[k],
                                         start=(k == 0), stop=(k == KT1 - 1))
                    nc.scalar.activation(sc_sb[_b][m][:, n * CHUNK:(n + 1) * CHUNK], ps[:],
                                         ACT.Identity, bias=bsc[:, m:m + 1], scale=1.0)
            return f

        def sink1(b):
            def f(mo, n, src_ap, _b=b):
                vH = pads1H[_b][:].rearrange("p (k r c) -> p k r c", k=KT1, r=PW1)[
                    :, mo, 1 + 16 * n:1 + 16 * (n + 1), 1:33]
                vL = pads1L[_b][:].rearrange("p (k r c) -> p k r c", k=KT1, r=PW1)[
                    :, mo, 1 + 16 * n:1 + 16 * (n + 1), 1:33]
                rel = scr.tile([128, CHUNK], BF, name=f"rel1_{_b}_{mo}_{n}", tag="scr_rel", bufs=3)
                nc.vector.tensor_scalar_max(rel[:], src_ap, 0.0)
                sr = rel[:].rearrange("p (r c) -> p r c", r=16)
                nc.scalar.copy(vH, sr)
                nc.gpsimd.tensor_tensor(vL, sr, vH, OP.subtract)
            return f

        def load1(_b, n, tag):
            return [x_t[_b][:, k * P1 + n * CHUNK:k * P1 + (n + 1) * CHUNK] for k in range(KT1)]

        for b in range(B_LOC):
            for n in range(NCH1):
                stage_front(b, n, KT1, load1, wimg1, extra1(b), "s1", roff=b * NCH1)
        s1v, t1v = bn_post(ar1_out, KT1, B_STATS * P1, bn1w, bn1b, "bn1")
        for b in range(B_LOC):
            stage_mid(b, NCH1, "s1", roff=b * NCH1)
        for b in range(B_LOC):
            for n in range(NCH1):
                stage_back(b, n, wg1, wb1, A1, B1, s1v, t1v, load1, sink1(b),
                           gb_evac_acts=2, name="s1", roff=b * NCH1)

        # ================= conv1 (q-major, both batches) =================
        sum2P = cpool.tile([128, KT2 * B_LOC * 8], F32, name="sum2P")
        ss2P = cpool.tile([128, KT2 * B_LOC * 8], F32, name="ss2P")

        for q in range(4):
            a_, b2_ = q // 2, q % 2
            roff = [0, 1] if a_ == 0 else [1, 2]
            coff = [0, 1] if b2_ == 0 else [1, 2]
            w1qh = scr.tile([128, 4 * 2 * 2 * 2 * 128], F8, name=f"w1qh_{q}", tag="w1qh", bufs=2)
            nc.sync.dma_start(
                w1qh[:].rearrange("p (t kp mo s m) -> p t kp mo s m", t=4, kp=2, mo=2, s=2),
                w1h_d.ap()[q].rearrange("t kp mo p s m -> p t kp mo s m"))
            w1ql = scr.tile([128, 4 * 2 * 2 * 2 * 128], F8, name=f"w1ql_{q}", tag="w1ql", bufs=2)
            nc.sync.dma_start(
                w1ql[:].rearrange("p (t kp mo s m) -> p t kp mo s m", t=4, kp=2, mo=2, s=2),
                w1l_d.ap()[q].rearrange("t kp mo p s m -> p t kp mo s m"))

            def w1dr(ht, t, kp, mo):
                off = ((t * 2 + kp) * 2 + mo) * 256
                return ht[:, off:off + 256].rearrange("p (s m) -> p s m", s=2)
            for b in range(B_LOC):
                padH = pads1H[b][:].rearrange("p (k r c) -> p k r c", k=KT1, r=PW1)
                padL = pads1L[b][:].rearrange("p (k r c) -> p k r c", k=KT1, r=PW1)
                for m in range(KT2):
                    for n in range(NCH1):
                        pc = psum.tile([128, CHUNK], F32, name=f"pc1_{b}_{q}_{m}_{n}", tag="ps_mm", bufs=2)
                        DR = mybir.MatmulPerfMode.DoubleRow
                        for hf in range(2):
                            out_h = pc[:, hf * 256:(hf + 1) * 256]
                            first = True
                            for ti in range(4):
                                si, tj = ti // 2, ti % 2
                                r0 = 16 * n + 8 * hf + roff[si]
                                c0 = coff[tj]
                                for kp in range(2):
                                    rH = padH[:, 2 * kp:2 * kp + 2, r0:r0 + 8, c0:c0 + 32]
                                    rL = padL[:, 2 * kp:2 * kp + 2, r0:r0 + 8, c0:c0 + 32]
                                    nc.tensor.matmul(out_h, w1dr(w1qh, ti, kp, m), rH,
                                                     start=first, stop=False, perf_mode=DR)
                                    first = False
                                    nc.tensor.matmul(out_h, w1dr(w1qh, ti, kp, m), rL,
                                                     start=False, stop=False, perf_mode=DR)
                                    nc.tensor.matmul(out_h, w1dr(w1ql, ti, kp, m), rH,
                                                     start=False,
                                                     stop=(ti == 3 and kp == 1 and hf == 1),
                                                     perf_mode=DR)
                        ci = (m * B_LOC + b) * 8 + q * NCH1 + n
                        sp = scr.tile([128, CHUNK], BF, name=f"sp_{b}_{q}_{m}_{n}", tag="scr_sp", bufs=3)
                        nc.scalar.activation(sp[:], pc[:], ACT.Copy, scale=1.0 / 64.0,
                                             accum_out=sum2P[:, ci:ci + 1])
                        thr = scr.tile([128, CHUNK], BF, name=f"thr1_{b}_{q}_{m}_{n}", tag="scr_ic", bufs=4)
                        nc.vector.scalar_tensor_tensor(
                            thr[:], sp[:], 0.0, sp[:], OP.add, OP.mult,
                            accum_out=ss2P[:, ci:ci + 1])
                        nc.sync.dma_start(
                            out1_d[b, m * 128:(m + 1) * 128,
                                   q * P1 + n * CHUNK: q * P1 + (n + 1) * CHUNK], sp[:])

        # prefetch the first stage2 b0 front chunks while conv1 finishes
        pre2 = {}
        for nf in (0, 2, 4):
            outp = []
            for k in range(KT2):
                t = scr.tile([128, CHUNK], BF, name=f"o1f_0_{k}_{nf}",
                             tag="o1f", bufs=6)
                nc.sync.dma_start(t[:], out1_d[0, k * 128:(k + 1) * 128,
                                               nf * CHUNK:(nf + 1) * CHUNK])
                outp.append(t[:])
            pre2[nf] = outp

        # ================= BN2 AllReduce =================
        ar2_in = dram.tile([KT2, 128, 2], F32, name="ar2_in")
        ar2_out = dram.tile([KT2, 128, 2], F32, name="ar2_out",
                            addr_space="Shared" if use_collectives else "Local")
        st2 = cpool.tile([128, 2 * KT2], F32, name="st2")
        for m in range(KT2):
            nc.vector.tensor_reduce(st2[:, 2 * m:2 * m + 1],
                                    sum2P[:, m * 16:(m + 1) * 16], AX.X, OP.add)
            nc.vector.tensor_reduce(st2[:, 2 * m + 1:2 * m + 2],
                                    ss2P[:, m * 16:(m + 1) * 16], AX.X, OP.add)
            nc.sync.dma_start(ar2_in[m], st2[:, 2 * m:2 * m + 2])
        if use_collectives:
            nc.gpsimd.collective_compute(
                "AllReduce", OP.add, replica_groups=[list(range(num_devices))],
                ins=[ar2_in.opt()], outs=[ar2_out.opt()])
        else:
            nc.sync.dma_start(ar2_out[:], ar2_in[:])

        # free x + pads1; pads2 reuses that space
        p1pads_st.close()
        p2pads_st = contextlib.ExitStack()
        p2pads = p2pads_st.enter_context(tc.tile_pool(name="p2pads", bufs=1))
        pads2H = [p2pads.tile([128, KT2 * PW2 * PW2], F8, name=f"pad2H_{b}")
                  for b in range(B_LOC)]
        pads2L = [p2pads.tile([128, KT2 * PW2 * PW2], F8, name=f"pad2L_{b}")
                  for b in range(B_LOC)]
        for b in range(B_LOC):
            for k in range(KT2):
                zero_borders8(pads2H[b], k, KT2, PW2)
                zero_borders8(pads2L[b], k, KT2, PW2)

        # ================= stage 2 + conv2 =================
        order = [0, 2, 4, 6, 1, 3, 5, 7]
        s2v = t2v = None
        conv2_carry = []
        for b in range(B_LOC):
            def sink2(mo, n, src_ap, _b=b):
                qq, hh = n // 2, n % 2
                aq, bq = qq // 2, qq % 2
                r0 = 1 + aq + 32 * hh
                c0 = 1 + bq
                vH = pads2H[_b][:].rearrange("p (k r c) -> p k r c", k=KT2, r=PW2)[
                    :, mo, r0:r0 + 32:2, c0:c0 + 64:2]
                vL = pads2L[_b][:].rearrange("p (k r c) -> p k r c", k=KT2, r=PW2)[
                    :, mo, r0:r0 + 32:2, c0:c0 + 64:2]
                s = src_ap.rearrange("p (r c) -> p r c", r=16)
                rel = scr.tile([128, CHUNK], BF, name=f"rel2_{_b}_{mo}_{n}", tag="scr_rel", bufs=3)
                nc.vector.tensor_scalar_max(rel[:], src_ap, 0.0)
                nc.scalar.copy(vH, rel[:].rearrange("p (r c) -> p r c", r=16))
                nc.gpsimd.tensor_tensor(vL, rel[:].rearrange("p (r c) -> p r c", r=16),
                                        vH, OP.subtract)

            def load2(_b, n, tag):
                if _b == 0 and tag == "f" and n in pre2:
                    return pre2.pop(n)
                out = []
                for k in range(KT2):
                    t = scr.tile([128, CHUNK], BF, name=f"o1{tag}_{_b}_{k}_{n}",
                                 tag=f"o1{tag}", bufs=6)
                    nc.sync.dma_start(t[:], out1_d[_b, k * 128:(k + 1) * 128,
                                                   n * CHUNK:(n + 1) * CHUNK])
                    out.append(t[:])
                return out

            def conv2_tile(m, n, _b):
                pc = psum.tile([128, CHUNK], F32, name=f"pc2_{_b}_{m}_{n}", tag="ps_conv", bufs=2)
                padH = pads2H[_b][:].rearrange("p (k r c) -> p k r c", k=KT2, r=PW2)
                padL = pads2L[_b][:].rearrange("p (k r c) -> p k r c", k=KT2, r=PW2)
                for hf in range(2):
                    first = True
                    for t in range(9):
                        ku, kv = t // 3, t % 3
                        r0 = 8 * n + 4 * hf + ku
                        rH = padH[:, :, r0:r0 + 4, kv:kv + 64]
                        rL = padL[:, :, r0:r0 + 4, kv:kv + 64]
                        out_h = pc[:, hf * 256:(hf + 1) * 256]
                        DR = mybir.MatmulPerfMode.DoubleRow
                        nc.tensor.matmul(out_h, w2dr(w2h_t, t, m), rH,
                                         start=first, stop=False, perf_mode=DR)
                        first = False
                        nc.tensor.matmul(out_h, w2dr(w2h_t, t, m), rL,
                                         start=False, stop=False, perf_mode=DR)
                        nc.tensor.matmul(out_h, w2dr(w2l_t, t, m), rH,
                                         start=False, stop=(t == 8), perf_mode=DR)
                fin = scr.tile([128, CHUNK], F32, name=f"fin_{_b}_{m}_{n}", tag="scr_fin", bufs=2)
                scv4 = sc_sb[_b][m][:].rearrange("p (i j) -> p i j", i=32)[
                    :, 4 * n:4 * n + 4, :].unsqueeze(3).to_broadcast((128, 4, 32, 2))
                for a_ in (0, 1):
                    nc.vector.scalar_tensor_tensor(
                        fin[:].rearrange("p (i a j c) -> p i a j c", i=4, a=2, j=32)[:, :, a_],
                        pc[:].rearrange("p (i a j c) -> p i a j c", i=4, a=2, j=32)[:, :, a_],
                        1.0 / 64.0, scv4, OP.mult, OP.add)
                nc.sync.dma_start(out_d.ap()[_b, m * 128:(m + 1) * 128, n * CHUNK:(n + 1) * CHUNK],
                                  fin[:])

            for i, n in enumerate(order):
                stage_front(b, n, KT2, load2, wimg2, None, "s2")
                if conv2_carry:
                    fb, fm, fn = conv2_carry.pop(0)
                    conv2_tile(fm, fn, fb)
            while conv2_carry:
                fb, fm, fn = conv2_carry.pop(0)
                conv2_tile(fm, fn, fb)
            if b == 0:
                s2v, t2v = bn_post(ar2_out, KT2, B_STATS * P2, bn2w, bn2b, "bn2")
            stage_mid(b, NCH2, "s2")
            ready = []
            for i, n in enumerate(order):
                stage_back(b, n, wg2, wb2, A2, B2, s2v, t2v, load2, sink2,
                           gb_evac_acts=1, name="s2")
                if i == 3:
                    # rows < 27 complete (hh=0 planes): conv2 tiles 0..2
                    ready = [(m, nc_) for m in range(KT2) for nc_ in range(3)]
                for _ in range(2):
                    if ready:
                        fm, fn = ready.pop(0)
                        conv2_tile(fm, fn, b)
            while ready:
                fm, fn = ready.pop(0)
                conv2_tile(fm, fn, b)
            conv2_carry = [(b, m, nc_) for m in range(KT2) for nc_ in range(3, NCH2)]
        while conv2_carry:
            fb, fm, fn = conv2_carry.pop(0)
            conv2_tile(fm, fn, fb)
        p2pads_st.close()
        psum_st.close()
        p1b.close()
        st.close()

    nc.compile()
    return nc


# ---------------------------------------------------------------------------
# host side
# ---------------------------------------------------------------------------
_cached = {}


def _bf(a):
    import ml_dtypes
    return np.ascontiguousarray(np.asarray(a, np.float32).astype(ml_dtypes.bfloat16))


def _f32(a):
    return np.ascontiguousarray(np.asarray(a, np.float32))


def _prep_weights(inputs):
    w = {}
    w["w_img1T"] = _bf(inputs["w_img1"].T)
    w["w_img2T"] = _bf(inputs["w_img2"].T)
    w["w_g1T"] = _bf(inputs["w_g1"].T * 16.0)
    w["w_b1T"] = _bf(inputs["w_b1"].T * 16.0)
    w["w_g2T"] = _bf(inputs["w_g2"].T * 16.0)
    w["w_b2T"] = _bf(inputs["w_b2"].T * 16.0)
    for k in ("bn1_w", "bn1_b", "bn2_w", "bn2_b", "b_sc"):
        w[k] = _f32(inputs[k])
    for k in ("b_g1", "b_b1", "b_g2", "b_b2"):
        w[k] = _f32(np.asarray(inputs[k], np.float32) * 16.0)
    w["w_scT"] = _bf(inputs["w_sc"][:, :, 0, 0].T)

    import ml_dtypes
    F8NP_ = ml_dtypes.float8_e4m3
    wc1 = np.asarray(inputs["w_c1"], np.float32)
    rows = {0: [[0], [1, 2]], 1: [[0, 1], [2]]}
    w1sub = np.zeros((4, IN, 4, OUT), np.float32)   # [q][ic][tap][o]
    for a in (0, 1):
        for b2 in (0, 1):
            q = a * 2 + b2
            for si in (0, 1):
                for tj in (0, 1):
                    acc = np.zeros((OUT, IN), np.float32)
                    for ku in rows[a][si]:
                        for kv in rows[b2][tj]:
                            acc += wc1[:, :, ku, kv]
                    w1sub[q, :, si * 2 + tj, :] = acc.T
    w1q_ = w1sub * 4.0                               # 64/16
    h1 = w1q_.astype(F8NP_).astype(np.float32)
    l1 = (w1q_ - h1).astype(F8NP_).astype(np.float32)
    # [q][ic][t][o] -> [q][t][kp][mo][c][s][m]; ic = (kp*2+s)*128 + c
    def drpack1(a):
        r = a.reshape(4, 2, 2, 128, 4, 2, 128)       # [q][kp][s][c][t][mo][m]
        return np.ascontiguousarray(np.transpose(r, (0, 4, 1, 5, 3, 2, 6)).astype(F8NP_))
    w["w1drh"] = drpack1(h1)
    w["w1drl"] = drpack1(l1)
    import ml_dtypes
    F8NP = ml_dtypes.float8_e4m3
    wc2 = np.asarray(inputs["w_c2"], np.float32)
    w2taps = np.zeros((9, OUT, OUT), np.float32)
    for t in range(9):
        w2taps[t] = wc2[:, :, t // 3, t % 3].T          # [t][ic][o]
    wq = w2taps * 4.0                                   # 64/16: pads carry x16
    hi = wq.astype(F8NP).astype(np.float32)
    lo = (wq - hi).astype(F8NP).astype(np.float32)
    # [t][ic][o] -> [t][mo][c][slot][m]; slot = ic-tile, c = ic within tile
    def drpack(a):
        r = a.reshape(9, 2, 128, 2, 128)                # [t][s][c][mo][m]
        return np.ascontiguousarray(np.transpose(r, (0, 3, 2, 1, 4)).astype(F8NP))
    w["w2drh"] = drpack(hi)
    w["w2drl"] = drpack(lo)
    return w


def make_in_maps(inputs):
    w = _prep_weights(inputs)
    x = np.asarray(inputs["x"], np.float32).reshape(B, IN, P1)
    gc = np.asarray(inputs["global_cond"], np.float32)
    words = np.asarray(inputs["words_embs"], np.float32)
    wn = np.sqrt((words * words).sum(axis=1, keepdims=True))
    wordsn = words / np.maximum(wn, 1e-12)
    maskb = np.asarray(inputs["mask"]).astype(np.float32) * MASK_BIAS
    in_maps = []
    for c in range(N_CORES):
        sl = slice(c * B_LOC, (c + 1) * B_LOC)
        m = dict(w)
        m["x"] = _bf(x[sl])
        m["gcT"] = _bf(gc[sl].T)
        m["wordsn"] = _bf(wordsn[sl])
        m["wordsTn"] = _bf(wordsn[sl].transpose(0, 2, 1))
        m["maskb"] = _f32(maskb[sl])
        in_maps.append(m)
    return in_maps


def kernel(**inputs):
    from concourse.bass_utils import run_bass_kernel_spmd
    if "nc" not in _cached:
        _cached["nc"] = build_program()
    nc = _cached["nc"]
    in_maps = make_in_maps(inputs)
    res = run_bass_kernel_spmd(nc, in_maps, core_ids=list(range(N_CORES)))
    out = np.empty((B, OUT, 2 * H, 2 * W), np.float32)
    for c in range(N_CORES):
        out[c * B_LOC:(c + 1) * B_LOC] = res.results[c]["out"].reshape(B_LOC, OUT, 2 * H, 2 * W)
    return out
